# revision 1
# baseline (speedup 1.0000x reference)
"""Trainium2 Bass kernel for a 2-layer spiking (LIF) MLP scan.

Model (per reference):
  cur1 = x @ W1.T + b1           [B, 25]   (constant over time)
  25 timesteps of:
    reset1 = H(mem1 - 1); mem1 = 0.95*mem1 + cur1 - reset1; spk1 = H(mem1 - 1)
    cur2 = spk1 @ W2.T + b2
    reset2 = H(mem2 - 1); mem2 = 0.95*mem2 + cur2 - reset2; spk2 = H(mem2 - 1)
  outputs: spk2_rec, mem2_rec  each [25, B, 10]

Distribution: pure data parallel over 8 NeuronCores; batch 32768 padded to
32800 = 8 cores x 4100.

Device formulation: doubled units M = 2*mem and sign-coded spikes
sigma = 2*spk - 1 in {-1, +1}:
    M_t = beta*M_{t-1} + chat - sigma_{t-1},   chat = 2*cur - 1
    sigma_t = Sign(M_t - 2)            (ScalarE activation - frees DVE)
  with sigma_init = -1, M_init = 0. Layer-2 drive is a block-diagonal W2
  matmul over sigma1 plus a tiny K=2 matmul of the constant row
  sum_k W2[j,k] + 2*b2[j] - 1 against an all-ones tile. The host halves M2
  and thresholds sigma2 > 0 to recover mem2/spk2 exactly.

Matmul precision: fp16 split-accumulate. x = xa + xb (two fp16 terms holds
~22 of f32's 24 mantissa bits), W = wa + wb likewise; accumulating
xa@wa + xa@wb + xb@wa in fp32 PSUM gives ~f32-class results at 1 cycle/row
(vs 4 cycles/row for native f32 matmul). sigma is exactly representable in
fp16, so the layer-2 matmul needs only the weight split (2 terms).

Per-core layout:
  - x arrives host-transposed as [784, 4100] fp16 split pairs; the layer-1
    bias (2*b1 - 1) is applied as a per-partition ScalarE bias during the
    PSUM->SBUF copy of the GEMM result.
  - Layer-1 state is feature-packed: 5 batch groups x 25 features = 125
    partitions, 820 batch columns -> [125, 820] tiles.
  - Layer-2 state is batch-packed [100, 410]: row 50*h + 10*g + j holds
    feature j of batch element g*820 + h*410 + n.
  - Scheduling: engines run their streams in order, so the batch-half-0
    layer-1 recurrence is emitted as one contiguous block that overlaps the
    half-1 GEMM; a 25-deep sigma ring decouples it from the PE-ordered
    layer-2 matmuls. GpSimd takes a column-split share of the spike
    subtracts in the tail phase.
"""

import numpy as np

BETA = 0.95
T = 25
B_FULL = 32768
D = 784
H1 = 25
H2 = 10
N_CORES = 8
BC = 4100          # per-core batch (padded)
G = 5              # feature-packing groups
COLS = BC // G     # 820 batch columns per group
HALF = COLS // 2   # 410
P1 = G * H1        # 125 partitions for layer-1 state
P2 = 2 * G * H2    # 100 rows of the layer-2 tile

KC = 7           # k-chunks of 112 rows: 7*112 = 784
KSZ = D // KC

# columns (of each 410-col half) whose layer-1 spike-subtract runs on GpSimd
POOL_HALF = 288
SIG_RING = 25      # sigma1 tiles: one per step (full cross-step pipelining)
L2_POOL = 300      # layer-2 subtract columns handled by gpsimd

GEMM_MODE = "f16x3"    # 'f32' | 'f16x3'
SCANMM_MODE = "f16x2"  # 'f32' | 'f16x2'

_CACHED = {}


def _build_program(variant="v3"):
    from contextlib import ExitStack

    import concourse.bacc as bacc
    import concourse.tile as tile
    from concourse import mybir

    dt = mybir.dt
    alu = mybir.AluOpType
    act_fn = mybir.ActivationFunctionType

    nc = bacc.Bacc(
        "TRN2",
        target_bir_lowering=False,
        debug=False,
        enable_asserts=False,
        num_devices=N_CORES,
    )

    f16 = dt.float16
    f32 = dt.float32

    if GEMM_MODE == "f16x3":
        xa_d = nc.dram_tensor("xa", [D, BC], f16, kind="ExternalInput").ap()
        xb_d = nc.dram_tensor("xb", [D, BC], f16, kind="ExternalInput").ap()
        w1a_d = nc.dram_tensor("w1a", [D, G * P1], f16, kind="ExternalInput").ap()
        w1b_d = nc.dram_tensor("w1b", [D, G * P1], f16, kind="ExternalInput").ap()
    else:
        xT_d = nc.dram_tensor("xT", [D, BC], f32, kind="ExternalInput").ap()
        w1t_d = nc.dram_tensor("w1t", [D, G * P1], f32, kind="ExternalInput").ap()

    smm_dt = f16 if SCANMM_MODE == "f16x2" else f32
    # per-partition GEMM bias (2*b1[j] - 1 at row 25g+j)
    cb1_d = nc.dram_tensor("cb1", [P1, 1], f32, kind="ExternalInput").ap()
    # the layer-2 constant (sum_k W2 + 2*b2 - 1) enters via a tiny K=2 matmul
    # of split bias rows against a constant ones tile
    ones2_d = nc.dram_tensor("ones2", [2, HALF], smm_dt, kind="ExternalInput").ap()
    wbdc_d = nc.dram_tensor("wbdc", [2, P2], smm_dt, kind="ExternalInput").ap()
    n_wbd = 4 if SCANMM_MODE == "f16x2" else 2
    wbd_d = nc.dram_tensor(
        "wbd", [P1, n_wbd * P2], smm_dt, kind="ExternalInput"
    ).ap()

    # merged per-step record: cols 0:410 = M2, cols 410:820 = sigma2
    rec_out = nc.dram_tensor(
        "rec_out", [T, P2, COLS], f32, kind="ExternalOutput"
    ).ap()

    with tile.TileContext(nc) as tc, ExitStack() as ctx:
        w1_pool = ctx.enter_context(tc.tile_pool(name="w1", bufs=1))
        wbd_pool = ctx.enter_context(tc.tile_pool(name="wbd", bufs=1))
        xin_pool = ctx.enter_context(tc.tile_pool(name="xin", bufs=6))
        state_pool = ctx.enter_context(tc.tile_pool(name="state", bufs=1))
        l2_pool = ctx.enter_context(tc.tile_pool(name="l2", bufs=6))
        psA_pool = ctx.enter_context(tc.tile_pool(name="psA", bufs=2, space="PSUM"))
        ps2_pool = ctx.enter_context(tc.tile_pool(name="ps2", bufs=5, space="PSUM"))

        # --- weights: one [112, 7*625] tile per split term ---
        w1_tiles = []
        w1_srcs = (
            [(w1a_d, "w1a"), (w1b_d, "w1b")]
            if GEMM_MODE == "f16x3"
            else [(w1t_d, "w1t")]
        )
        w1dt = f16 if GEMM_MODE == "f16x3" else f32
        # per-chunk DMAs so the first matmuls can start as early as possible
        for src_d, tag in w1_srcs:
            wt = w1_pool.tile([KSZ, KC * G * P1], w1dt, tag=tag)
            w1_tiles.append(wt)
        # weights ride the ScalarE HWDGE queue so the first x tiles aren't
        # stuck behind them on the sync queue
        for k in range(KC):
            for (src_d, tag), wt in zip(w1_srcs, w1_tiles):
                nc.scalar.dma_start(
                    wt[:, k * G * P1 : (k + 1) * G * P1],
                    src_d[k * KSZ : (k + 1) * KSZ, :],
                )

        wbd_tiles = []
        for i in range(n_wbd):
            wt = wbd_pool.tile([P1, P2], smm_dt, tag=f"wbd{i}")
            nc.scalar.dma_start(wt[:], wbd_d[:, i * P2 : (i + 1) * P2])
            wbd_tiles.append(wt)
        wbdc = wbd_pool.tile([2, P2], smm_dt, tag="wbdc")
        nc.scalar.dma_start(wbdc[:], wbdc_d[:])
        ones2 = wbd_pool.tile([2, HALF], smm_dt, tag="ones2")
        nc.scalar.dma_start(ones2[:], ones2_d[:])

        # --- persistent state ---
        chat1 = state_pool.tile([P1, COLS], f32, tag="chat1")
        mh1A = state_pool.tile([P1, COLS], f32, tag="mh1A")
        mh1B = state_pool.tile([P1, COLS], f32, tag="mh1B")
        # sigma ring: one tile per step (written fully by Sign before reads)
        sig_ring = []
        for i in range(SIG_RING):
            sg = state_pool.tile([P1, COLS], smm_dt, tag=f"sig{i}")
            sig_ring.append(sg)
        sig_init = state_pool.tile([P1, COLS], smm_dt, tag="sig_init")
        nc.vector.memset(sig_init[:], -1.0)
        nc.vector.memset(mh1B[:], 0.0)
        cb1 = state_pool.tile([P1, 1], f32, tag="cb1")
        nc.sync.dma_start(cb1[:], cb1_d[:])

        rec0 = l2_pool.tile([P2, COLS], f32, tag="rec")
        nc.vector.memset(rec0[:], 0.0)
        nc.vector.memset(rec0[:, HALF:COLS], -1.0)
        mh2_prev = rec0[:, 0:HALF]
        s2_prev = rec0[:, HALF:COLS]

        # per-partition bias column (-2.0) for the Sign activations
        biasc = state_pool.tile([128, 1], f32, tag="biasc")
        nc.vector.memset(biasc[:], -2.0)

        # --- main GEMM -> chat1 = 2*cur1 - 1, feature-packed [125, 820] ---
        # One k-blocked DMA per (g, h) per x-term: [112, 7*410] tiles.
        xdt = f16 if GEMM_MODE == "f16x3" else f32
        x_srcs = [xa_d, xb_d] if GEMM_MODE == "f16x3" else [xT_d]
        for h in range(2):
            ps = psA_pool.tile([P1, HALF], f32)
            first = True
            for g in range(G):
                col0 = g * COLS + h * HALF
                xts = []
                for xi, src_d in enumerate(x_srcs):
                    xt = xin_pool.tile([KSZ, KC * HALF], xdt, tag=f"x{xi}")
                    nc.sync.dma_start(
                        xt[:].rearrange("p (c n) -> p c n", c=KC),
                        src_d.rearrange("(c p) n -> p c n", p=KSZ)[
                            :, :, col0 : col0 + HALF
                        ],
                    )
                    xts.append(xt)
                if GEMM_MODE == "f16x3":
                    terms = [(0, 0), (1, 0), (0, 1)]  # (w term, x term)
                else:
                    terms = [(0, 0)]
                for k in range(KC):
                    last_k = g == G - 1 and k == KC - 1
                    for j, (wi, xi) in enumerate(terms):
                        nc.tensor.matmul(
                            ps[:],
                            lhsT=w1_tiles[wi][
                                :, k * G * P1 + g * P1 : k * G * P1 + (g + 1) * P1
                            ],
                            rhs=xts[xi][:, k * HALF : (k + 1) * HALF],
                            start=first,
                            stop=last_k and j == len(terms) - 1,
                        )
                        first = False
            # bias (2*b1 - 1) folded into the PSUM->SBUF copy
            nc.scalar.activation(
                chat1[:, h * HALF : (h + 1) * HALF], ps[:],
                act_fn.Identity, bias=cb1[:], scale=1.0,
            )

        # --- the 25-step scan ---
        # Engines execute in order, so the h0 layer-1 recurrence is emitted as
        # one contiguous block: it only needs the GEMM's h0 output and can run
        # 25 steps deep while the h1 GEMM is still streaming. The h1 block and
        # the (batch-complete) layer-2 chain follow.
        def l1_step(t, h, pool_cols):
            prev = sig_ring[t - 1] if t > 0 else sig_init
            new = sig_ring[t]
            m_prev = mh1B if t % 2 == 0 else mh1A
            m_cur = mh1A if t % 2 == 0 else mh1B
            sl = slice(h * HALF, (h + 1) * HALF)
            nc.vector.scalar_tensor_tensor(
                m_cur[:, sl], m_prev[:, sl], BETA, chat1[:, sl],
                op0=alu.mult, op1=alu.add,
            )
            if pool_cols:
                pc = slice(h * HALF, h * HALF + pool_cols)
                nc.gpsimd.tensor_tensor(
                    m_cur[:, pc], m_cur[:, pc], prev[:, pc], op=alu.subtract
                )
            vc = slice(h * HALF + pool_cols, (h + 1) * HALF)
            nc.vector.tensor_tensor(
                m_cur[:, vc], m_cur[:, vc], prev[:, vc], op=alu.subtract
            )
            # sigma1 = Sign(M1 - 2)
            nc.scalar.activation(
                new[:, sl], m_cur[:, sl], act_fn.Sign,
                bias=biasc[0:P1, :], scale=1.0,
            )

        # h0 front-run: subtract stays on DVE (in-order, no cross-engine hop
        # in the recurrence) so the chain paces with the h1 GEMM stream
        for t in range(T):
            l1_step(t, 0, 0)

        mh2_prev = rec0[:, 0:HALF]
        s2_prev = rec0[:, HALF:COLS]
        for t in range(T):
            l1_step(t, 1, POOL_HALF)
            new = sig_ring[t]
            # layer-2 drive [100, 410]: bias matmul + blockdiag terms
            ps2 = ps2_pool.tile([P2, HALF], f32)
            if SCANMM_MODE == "f16x2":
                mms = [(0, 0), (1, 0), (2, 1), (3, 1)]  # (wbd idx, half)
            else:
                mms = [(0, 0), (1, 1)]
            nc.tensor.matmul(
                ps2[:], lhsT=wbdc[:], rhs=ones2[:], start=True, stop=False
            )
            for j, (wi, h) in enumerate(mms):
                nc.tensor.matmul(
                    ps2[:],
                    lhsT=wbd_tiles[wi][:],
                    rhs=new[:, h * HALF : (h + 1) * HALF],
                    start=False,
                    stop=(j == len(mms) - 1),
                )
            # M2 = beta*M2_prev + chat2; record tile holds [M2 | sigma2]
            rec = l2_pool.tile([P2, COLS], f32, tag="rec")
            mh2n = rec[:, 0:HALF]
            s2n = rec[:, HALF:COLS]
            nc.vector.scalar_tensor_tensor(
                mh2n, mh2_prev, BETA, ps2[:], op0=alu.mult, op1=alu.add
            )
            # M2 -= sigma2_prev (split pool/dve)
            nc.gpsimd.tensor_tensor(
                rec[:, 0:L2_POOL], rec[:, 0:L2_POOL],
                s2_prev[:, 0:L2_POOL], op=alu.subtract,
            )
            nc.vector.tensor_tensor(
                rec[:, L2_POOL:HALF], rec[:, L2_POOL:HALF],
                s2_prev[:, L2_POOL - HALF :], op=alu.subtract,
            )
            # sigma2 = Sign(M2 - 2)
            nc.scalar.activation(
                s2n, mh2n, act_fn.Sign, bias=biasc[0:P2, :], scale=1.0
            )
            nc.sync.dma_start(rec_out[t], rec[:])
            mh2_prev = mh2n
            s2_prev = s2n

    nc.compile()
    return nc


def _get_nc(variant="v3"):
    key = (variant, GEMM_MODE, SCANMM_MODE)
    if key not in _CACHED:
        _CACHED[key] = _build_program(variant)
    return _CACHED[key]


def _f16_split(a):
    hi = a.astype(np.float16)
    lo = (a.astype(np.float32) - hi.astype(np.float32)).astype(np.float16)
    return hi, lo


def _host_inputs(x, W1, b1, W2, b2):
    ins = {}
    xp = np.zeros((D, N_CORES * BC), np.float32)
    xp[:, : x.shape[0]] = x.T
    # chat1 = x @ (2*W1).T + (2*b1 - 1): block-column layout; the bias part
    # is applied on-device via the per-partition cb1 column.
    w1blocks = np.zeros((D, G * P1), np.float32)
    for g in range(G):
        w1blocks[:, P1 * g + H1 * g : P1 * g + H1 * (g + 1)] = 2.0 * W1.T
    ins["cb1"] = np.tile(2.0 * b1 - 1.0, G).astype(np.float32)[:, None]
    if GEMM_MODE == "f16x3":
        ins["xa"], ins["xb"] = _f16_split(xp)
        ins["w1a"], ins["w1b"] = _f16_split(w1blocks)
    else:
        ins["xT"] = xp
        ins["w1t"] = w1blocks
    # chat2 = sigma1 @ blockdiag(W2.T) + (sum_k W2[j,k] + 2*b2[j] - 1)
    bias2 = (W2.sum(axis=1) + 2.0 * b2 - 1.0).astype(np.float32)
    wbdf = np.zeros((P1, 2 * P2), np.float32)
    for blk, h in ((0, 0), (1, 1)):
        off = blk * P2 + h * G * H2
        for g in range(G):
            wbdf[g * H1 : (g + 1) * H1, off + g * H2 : off + (g + 1) * H2] = W2.T
    # bias_row[0, 50h+10g+j] = bias2[j]
    bias_row = np.tile(bias2, 2 * G)[None, :]
    smm_np = np.float16 if SCANMM_MODE == "f16x2" else np.float32
    ins["ones2"] = np.ones((2, HALF), smm_np)
    if SCANMM_MODE == "f16x2":
        A, B = wbdf[:, :P2], wbdf[:, P2:]
        Aa, Ab = _f16_split(A)
        Ba, Bb = _f16_split(B)
        # tile order [0..3] = Aa, Ab, Ba, Bb pairs with batch halves 0,0,1,1
        ins["wbd"] = np.concatenate([Aa, Ab, Ba, Bb], axis=1)
        Ca, Cb = _f16_split(bias_row)
        ins["wbdc"] = np.concatenate([Ca, Cb], axis=0)
    else:
        ins["wbd"] = wbdf
        ins["wbdc"] = np.concatenate([bias_row, np.zeros_like(bias_row)], axis=0)
    return ins


def kernel(x, W1, b1, W2, b2, _variant="v3", _trace=False, _tmpdir=None):
    from concourse.bass_utils import run_bass_kernel_spmd

    x = np.asarray(x, np.float32)
    W1 = np.asarray(W1, np.float32)
    b1 = np.asarray(b1, np.float32)
    W2 = np.asarray(W2, np.float32)
    b2 = np.asarray(b2, np.float32)
    B = x.shape[0]
    assert B == B_FULL, f"kernel hardcoded for B={B_FULL}, got {B}"

    full = _host_inputs(x, W1, b1, W2, b2)
    percore = [k for k in full if k in ("xa", "xb", "xT")]
    shared = {k: v for k, v in full.items() if k not in percore}
    in_maps = []
    for c in range(N_CORES):
        m = dict(shared)
        for k in percore:
            m[k] = np.ascontiguousarray(full[k][:, c * BC : (c + 1) * BC])
        in_maps.append(m)

    nc = _get_nc(_variant)
    res = run_bass_kernel_spmd(
        nc,
        in_maps,
        core_ids=list(range(N_CORES)),
        trace=_trace,
        tmpdir=_tmpdir,
    )

    spk = np.empty((T, N_CORES * BC, H2), np.float32)
    mem = np.empty((T, N_CORES * BC, H2), np.float32)
    for c in range(N_CORES):
        r = res.results[c]["rec_out"]  # [T, 100, 820]: [M2 | sigma2]
        for ci, dst in ((0, mem), (1, spk)):
            q = r[:, :, ci * HALF : (ci + 1) * HALF]
            q = q.reshape(T, 2, G, H2, HALF)  # [t, h, g, j, n]
            q = q.transpose(0, 2, 1, 4, 3)  # [t, g, h, n, j]
            dst[:, c * BC : (c + 1) * BC, :] = q.reshape(T, BC, H2)
    spk = (spk[:, :B_FULL, :] > 0.0).astype(np.float32)
    mem = mem[:, :B_FULL, :] * np.float32(0.5)
    kernel._last_results = res
    return spk, mem



# revision 37
# speedup vs baseline: 1.0793x; 1.0793x over previous
"""Trainium2 Bass kernel for a 2-layer spiking (LIF) MLP scan.

Model (per reference):
  cur1 = x @ W1.T + b1           [B, 25]   (constant over time)
  25 timesteps of:
    reset1 = H(mem1 - 1); mem1 = 0.95*mem1 + cur1 - reset1; spk1 = H(mem1 - 1)
    cur2 = spk1 @ W2.T + b2
    reset2 = H(mem2 - 1); mem2 = 0.95*mem2 + cur2 - reset2; spk2 = H(mem2 - 1)
  outputs: spk2_rec, mem2_rec  each [25, B, 10]

Distribution: pure data parallel over 8 NeuronCores; batch 32768 padded to
32800 = 8 cores x 4100.

Device formulation: doubled units M = 2*mem, sign-coded spikes
sigma = 2*spk - 1 in {-1, +1}:
    M_t = beta*M_{t-1} + chat - sigma_{t-1},   chat = 2*cur - 1
    sigma_t = Sign(M_t - 2)
with sigma_init = -1, M_init = 0.

Layer-2 runs bias-free via the change of variables M'' = M2 - cb2*s_t
(s_t = sum_{i<=t} beta^i, cb2 = sum_k W2[j,k] + 2*b2[j] - 1):
    M''_t = beta*M''_{t-1} + (sigma1_t @ bdW2) - sigma2_{t-1}
    sigma2_t = Sign(M''_t + (cb2*s_t - 2))     <- per-step ACT bias column
The -sigma2_{t-1} term is folded into the PE accumulation of the layer-2
drive via a (-I) matmul over the fp16 sigma2 ring, so the only elementwise
layer-2 op per step is one scalar_tensor_tensor from PSUM. Only M'' is
recorded; the host adds back cb2*s_t and thresholds to recover mem2/spk2
exactly (same fp32 compare as the device Sign).

Matmul precision: fp16 split-accumulate (xa+xb, wa+wb; terms aa, ba, ab)
for the input GEMM; layer-2 weights are 2-term fp16 (exact for +-1 sigma).

Per-core layout:
  - Layer-1 state is feature-packed: 5 batch groups x 25 features = 125
    partitions, 820 batch columns. The batch columns split into an "h0"
    block (C0 cols) and an "h1" block (820-C0): h0's GEMM (which needs only
    h0's slice of x) finishes early and its 25-step recurrence runs on the
    otherwise-idle DVE *during* the h1 GEMM's DMA window; sizes are chosen
    so both finish together.
  - x arrives host-packed per (group, block): one DMA of [112, 7*(2*Ch)]
    fp16 carrying xa|xb for all 7 k-chunks.
  - W1 block-diagonal tiles are built on device from a packed [112, 7*25]
    DMA per split term (memset + 5 strided copies) instead of shipping the
    5x-redundant zero-padded layout from DRAM.
  - Layer-2 state is batch-packed [100, 410]: row 50*h + 10*g + j holds
    feature j of batch element g*820 + h*410 + n.
  - Tail steady state per step: DVE [h1 stt, partial h1 subtract, lagged
    L2 stt], Pool [rest of h1 subtract], ACT [h1 Sign, lagged L2 Sign],
    PE [4 blockdiag-W2 matmuls + (-I) sigma2 matmul]. The L2 chain lags
    the h1 chain by LAG steps so cross-engine latency never stalls DVE.
  - The M'' record accumulates in SBUF ring tiles and is DMA'd out in
    REC_CH-step chunks.
"""

import numpy as np

BETA = 0.95
T = 25
B_FULL = 32768
D = 784
H1 = 25
H2 = 10
N_CORES = 8
BC = 4100          # per-core batch (padded)
G = 5              # feature-packing groups
COLS = BC // G     # 820 batch columns per group
HALF = COLS // 2   # 410 (layer-2 packing split, fixed)
P1 = G * H1        # 125 partitions for layer-1 state
P2 = 2 * G * H2    # 100 rows of the layer-2 tile

KC = 7             # k-chunks of 112 rows: 7*112 = 784
KSZ = D // KC

C0 = 320           # h0 block columns (h1 block = COLS - C0)
C1 = COLS - C0
SUB_D = 230        # h1 subtract: columns on DVE (rest on Pool; kept small
                   # so the sigma-loop sub->Sign->sub stays short)
LAG = 2            # L2 chain lag (steps) behind the h1 chain
REC_CH = 5         # record-chunk steps per output DMA (must divide T)
SIG_RING = 25      # sigma1 tiles: one per step
S2_RING = 4        # sigma2 fp16 ring depth

_CACHED = {}


def _build_program(variant="v4"):
    from contextlib import ExitStack

    import concourse.bacc as bacc
    import concourse.tile as tile
    from concourse import mybir

    dt = mybir.dt
    alu = mybir.AluOpType
    act_fn = mybir.ActivationFunctionType

    nc = bacc.Bacc(
        "TRN2",
        target_bir_lowering=False,
        debug=False,
        enable_asserts=False,
        num_devices=N_CORES,
    )

    f16 = dt.float16
    f32 = dt.float32

    # --- DRAM tensors ---
    # x packed per (g, block): [D, G*(2*C0 + 2*C1)] with, per group g,
    # cols [g*2*COLS, g*2*COLS + 2*C0) = xa|xb of block 0 and the rest
    # xa|xb of block 1.
    xq_d = nc.dram_tensor("xq", [D, 2 * G * COLS], f16, kind="ExternalInput").ap()
    # packed W1 content: per term i, [112, 7*25] (k-chunk-major)
    w1pk_d = nc.dram_tensor("w1pk", [KSZ, 2 * KC * H1], f16,
                            kind="ExternalInput").ap()
    cb1_d = nc.dram_tensor("cb1", [P1, 1], f32, kind="ExternalInput").ap()
    # blockdiag W2 split terms: Aa|Ab|Ba|Bb, each [P1, P2]
    wbd_d = nc.dram_tensor("wbd", [P1, 4 * P2], f16, kind="ExternalInput").ap()
    negI_d = nc.dram_tensor("negI", [P2, P2], f16, kind="ExternalInput").ap()
    th2_d = nc.dram_tensor("th2", [P2, T], f32, kind="ExternalInput").ap()

    mrec_out = nc.dram_tensor("mrec", [T, P2, HALF], f32,
                              kind="ExternalOutput").ap()

    blocks = [(0, C0), (C0, C1)]  # (col offset, width) per l1 block

    with tile.TileContext(nc) as tc, ExitStack() as ctx:
        w1_pool = ctx.enter_context(tc.tile_pool(name="w1", bufs=1))
        const_pool = ctx.enter_context(tc.tile_pool(name="cst", bufs=1))
        xin0_pool = ctx.enter_context(tc.tile_pool(name="xin0", bufs=5))
        xin1_pool = ctx.enter_context(tc.tile_pool(name="xin1", bufs=3))
        state_pool = ctx.enter_context(tc.tile_pool(name="state", bufs=1))
        rec_pool = ctx.enter_context(tc.tile_pool(name="rec", bufs=3))
        psA0_pool = ctx.enter_context(tc.tile_pool(name="psA0", bufs=1,
                                                   space="PSUM"))
        psA1_pool = ctx.enter_context(tc.tile_pool(name="psA1", bufs=2,
                                                   space="PSUM"))
        ps2_pool = ctx.enter_context(tc.tile_pool(name="ps2", bufs=5,
                                                  space="PSUM"))

        # cb1 + packed w1 ride first on the sync queue, ahead of the x
        # tiles; everything on this one queue transfers in priority order.
        cb1 = const_pool.tile([P1, 1], f32, tag="cb1", name="cb1")
        nc.sync.dma_start(cb1[:], cb1_d[:])
        w1pk = w1_pool.tile([KSZ, 2 * KC * H1], f16, tag="w1pk", name="w1pk")
        nc.sync.dma_start(w1pk[:], w1pk_d[:])

        # --- W1 block-diagonal tiles built on device: zero-fill on the
        # otherwise-idle Pool (tile 0) and ACT (tile 1, uint32 memzero so
        # uninitialized NaNs can't leak through), then 10 strided DVE
        # copies place the content at offset 150g of each 625-col k-block.
        w1_tiles = []
        wt0 = w1_pool.tile([KSZ, KC * G * P1], f16, tag="w1t0", name="w1t0")
        nc.gpsimd.memset(wt0[:], 0.0)
        w1_tiles.append(wt0)
        wt1 = w1_pool.tile([KSZ, KC * G * P1 + 1], f16, tag="w1t1",
                           name="w1t1")
        nc.scalar.memzero(wt1[:])
        w1_tiles.append(wt1)
        for i in range(2):
            src = w1pk[:, i * KC * H1:(i + 1) * KC * H1].rearrange(
                "p (c q) -> p c q", c=KC)
            for g in range(G):
                off = g * P1 + g * H1
                dst = w1_tiles[i][:, :KC * G * P1].rearrange(
                    "p (c q) -> p c q", c=KC)[:, :, off:off + H1]
                nc.vector.tensor_copy(dst, src)

        # --- small constants (ACT HWDGE queue; only needed by the tail) ---
        wbd_tiles = []
        for i in range(4):
            wt = const_pool.tile([P1, P2], f16, tag=f"wbd{i}", name=f"wbd{i}")
            nc.scalar.dma_start(wt[:], wbd_d[:, i * P2:(i + 1) * P2])
            wbd_tiles.append(wt)
        negI = const_pool.tile([P2, P2], f16, tag="negI", name="negI")
        nc.scalar.dma_start(negI[:], negI_d[:])
        th2 = const_pool.tile([P2, T], f32, tag="th2", name="th2")
        nc.scalar.dma_start(th2[:], th2_d[:])

        # --- persistent state ---
        chat1 = state_pool.tile([P1, COLS], f32, tag="chat1", name="chat1")
        mh1A = state_pool.tile([P1, COLS], f32, tag="mh1A", name="mh1A")
        mh1B = state_pool.tile([P1, COLS], f32, tag="mh1B", name="mh1B")
        sig_ring = []
        for i in range(SIG_RING):
            sg = state_pool.tile([P1, COLS], f16, tag=f"sig{i}",
                                 name=f"sig{i}")
            sig_ring.append(sg)
        sig_init = state_pool.tile([P1, COLS], f16, tag="sig_init",
                                   name="sig_init")
        nc.vector.memset(sig_init[:], -1.0)
        nc.vector.memset(mh1B[:], 0.0)
        # per-partition bias column (-2.0) for the layer-1 Sign
        biasc = state_pool.tile([128, 1], f32, tag="biasc", name="biasc")
        nc.vector.memset(biasc[:], -2.0)

        s2_ring = []
        for i in range(S2_RING):
            sg = state_pool.tile([P2, HALF], f16, tag=f"s2_{i}",
                                 name=f"s2_{i}")
            s2_ring.append(sg)
        nc.vector.memset(s2_ring[S2_RING - 1][:], -1.0)  # sigma2 at t=-1
        mh2_init = state_pool.tile([P2, HALF], f32, tag="mh2i", name="mh2i")
        nc.vector.memset(mh2_init[:], 0.0)

        # record ring: chunks of REC_CH steps
        n_chunks = T // REC_CH
        rec_tiles = [
            rec_pool.tile([P2, REC_CH * HALF], f32, tag="rec", name=f"rec{i}")
            for i in range(3)
        ]

        def m2_slice(v):
            if v < 0:
                return mh2_init[:]
            tl = rec_tiles[(v // REC_CH) % 3]
            sl = v % REC_CH
            return tl[:, sl * HALF:(sl + 1) * HALF]

        # --- main GEMM -> chat1 = 2*cur1 - 1, feature-packed [125, 820] ---
        # A block wider than 512 fp32 PSUM columns is accumulated in two
        # sub-range PSUM tiles (a matmul output must fit one PSUM bank).
        def gemm_block(h):
            cofs, cw = blocks[h]
            pool = xin0_pool if h == 0 else xin1_pool
            ps_pool = psA0_pool if h == 0 else psA1_pool
            nsub = 1 if cw <= 512 else 2
            subs = []  # (ps, sub_ofs, sub_w)
            sofs = 0
            for s in range(nsub):
                sw = cw // nsub + (1 if s < cw % nsub else 0)
                ps = ps_pool.tile([P1, sw], f32, tag=f"psA{h}",
                                  name=f"psA{h}_{s}")
                subs.append((ps, sofs, sw))
                sofs += sw
            for g in range(G):
                xt = pool.tile([KSZ, KC * 2 * cw], f16, tag="x",
                               name=f"x{h}g{g}")
                src0 = g * 2 * COLS + (0 if h == 0 else 2 * C0)
                nc.sync.dma_start(
                    xt[:].rearrange("p (c n) -> p c n", c=KC),
                    xq_d.rearrange("(c p) n -> p c n", p=KSZ)[
                        :, :, src0:src0 + 2 * cw],
                )
                for k in range(KC):
                    first = g == 0 and k == 0
                    last_k = g == G - 1 and k == KC - 1
                    for j, (wi, xo) in enumerate(((0, 0), (1, 0), (0, cw))):
                        lhsT = w1_tiles[wi][
                            :, k * G * P1 + g * P1:k * G * P1 + (g + 1) * P1]
                        for ps, sofs_, sw in subs:
                            nc.tensor.matmul(
                                ps[:],
                                lhsT=lhsT,
                                rhs=xt[:, k * 2 * cw + xo + sofs_:
                                       k * 2 * cw + xo + sofs_ + sw],
                                start=first and j == 0,
                                stop=last_k and j == 2,
                            )
            for ps, sofs_, sw in subs:
                nc.scalar.activation(
                    chat1[:, cofs + sofs_:cofs + sofs_ + sw], ps[:],
                    act_fn.Identity, bias=cb1[:], scale=1.0,
                )

        # PE warm-up: dummy matmuls from t~0 so the p-state ramp (3us of
        # continuous activity -> 2.4 GHz) completes before the first real
        # GEMM matmul instead of during it.
        ps_warm = ps2_pool.tile([P1, 64], f32, tag="ps2", name="ps_warm")
        for i in range(60):
            nc.tensor.matmul(
                ps_warm[:, 0:64],
                lhsT=sig_init[:, 0:P1],
                rhs=sig_init[:, 0:64],
                start=True,
                stop=True,
            )

        gemm_block(0)

        # --- layer-1 recurrence step ---
        # The DVE and Pool column ranges run as fully independent chains
        # (each engine does stt then subtract on its own columns, in order,
        # with no cross-engine hop in the recurrence); only the Sign joins
        # them, and the sigma ring gives the next subtract a full period of
        # slack before it needs the result.
        def l1_step(t, h, dve_cols):
            cofs, cw = blocks[h]
            prev = sig_ring[t - 1] if t > 0 else sig_init
            new = sig_ring[t]
            m_prev = mh1B if t % 2 == 0 else mh1A
            m_cur = mh1A if t % 2 == 0 else mh1B
            sl = slice(cofs, cofs + cw)
            nc.vector.scalar_tensor_tensor(
                m_cur[:, sl], m_prev[:, sl], BETA, chat1[:, sl],
                op0=alu.mult, op1=alu.add,
            )
            dc = slice(cofs, cofs + dve_cols)
            nc.vector.tensor_tensor(
                m_cur[:, dc], m_cur[:, dc], prev[:, dc], op=alu.subtract
            )
            if dve_cols < cw:
                pc = slice(cofs + dve_cols, cofs + cw)
                nc.gpsimd.tensor_tensor(
                    m_cur[:, pc], m_cur[:, pc], prev[:, pc], op=alu.subtract
                )
            nc.scalar.activation(
                new[:, sl], m_cur[:, sl], act_fn.Sign,
                bias=biasc[0:P1, :], scale=1.0,
            )

        # h0 chain: fully emitted first; runs during the h1 GEMM DMA window.
        # The h1 GEMM block is emitted after it so ACT's chat1-h1 copy does
        # not block the h0 Sign stream (ACT executes in order).
        def warm_pack(n, name):
            pw = ps2_pool.tile([P1, 64], f32, tag="ps2", name=name)
            for i in range(n):
                nc.tensor.matmul(
                    pw[:, 0:64], lhsT=sig_init[:, 0:P1],
                    rhs=sig_init[:, 0:64],
                    start=True, stop=True,
                )

        for t in range(T):
            l1_step(t, 0, C0)
        warm_pack(20, "warm_mid")
        gemm_block(1)
        warm_pack(16, "warm_gap")

        # --- tail: h1 chain + in-step layer-2 chain ---
        L2D = 140  # sigma2-subtract columns on DVE (rest on Pool)
        for u in range(T):
            l1_step(u, 1, SUB_D)
            new = sig_ring[u]
            ps2 = ps2_pool.tile([P2, HALF], f32, tag="ps2", name=f"ps2_{u}")
            for j, (wi, h) in enumerate(((0, 0), (1, 0), (2, 1), (3, 1))):
                nc.tensor.matmul(
                    ps2[:],
                    lhsT=wbd_tiles[wi][:],
                    rhs=new[:, h * HALF:(h + 1) * HALF],
                    start=(j == 0),
                    stop=(j == 3),
                )
            mh2n = m2_slice(u)
            s2p = (s2_ring[(u - 1) % S2_RING] if u > 0
                   else s2_ring[S2_RING - 1])
            nc.vector.scalar_tensor_tensor(
                mh2n, m2_slice(u - 1), BETA, ps2[:],
                op0=alu.mult, op1=alu.add,
            )
            nc.vector.tensor_tensor(
                mh2n[:, 0:L2D], mh2n[:, 0:L2D], s2p[:, 0:L2D],
                op=alu.subtract,
            )
            nc.gpsimd.tensor_tensor(
                mh2n[:, L2D:HALF], mh2n[:, L2D:HALF], s2p[:, L2D:HALF],
                op=alu.subtract,
            )
            nc.scalar.activation(
                s2_ring[u % S2_RING][:], mh2n, act_fn.Sign,
                bias=th2[:, u:u + 1], scale=1.0,
            )
            if u % REC_CH == REC_CH - 1:
                c = u // REC_CH
                tl = rec_tiles[c % 3]
                nc.sync.dma_start(
                    mrec_out[c * REC_CH:(c + 1) * REC_CH].rearrange(
                        "c p n -> p c n"),
                    tl[:].rearrange("p (c n) -> p c n", c=REC_CH),
                )

    nc.compile()
    return nc


def _get_nc(variant="v4"):
    if variant not in _CACHED:
        _CACHED[variant] = _build_program(variant)
    return _CACHED[variant]


def _f16_split(a):
    hi = a.astype(np.float16)
    lo = (a.astype(np.float32) - hi.astype(np.float32)).astype(np.float16)
    return hi, lo


def _host_inputs(x, W1, b1, W2, b2):
    ins = {}
    xp = np.zeros((D, N_CORES * BC), np.float32)
    xp[:, : x.shape[0]] = x.T
    xa, xb = _f16_split(xp)
    # pack per core c, group g: [xa|xb of block0, xa|xb of block1]
    xq = np.empty((D, N_CORES, G, 2 * COLS), np.float16)
    for c in range(N_CORES):
        for g in range(G):
            base = c * BC + g * COLS
            xq[:, c, g, 0:C0] = xa[:, base:base + C0]
            xq[:, c, g, C0:2 * C0] = xb[:, base:base + C0]
            xq[:, c, g, 2 * C0:2 * C0 + C1] = xa[:, base + C0:base + COLS]
            xq[:, c, g, 2 * C0 + C1:] = xb[:, base + C0:base + COLS]
    ins["xq"] = xq  # [D, N_CORES, G, 2*COLS]

    w1d = (2.0 * W1.T).astype(np.float32)  # [784, 25]
    wa, wb = _f16_split(w1d)
    w1pk = np.empty((KSZ, 2 * KC * H1), np.float16)
    for i, w in enumerate((wa, wb)):
        # [784, 25] -> [7, 112, 25] -> [112, 7*25]
        w1pk[:, i * KC * H1:(i + 1) * KC * H1] = (
            w.reshape(KC, KSZ, H1).transpose(1, 0, 2).reshape(KSZ, KC * H1))
    ins["w1pk"] = w1pk
    ins["cb1"] = np.tile(2.0 * b1 - 1.0, G).astype(np.float32)[:, None]

    # blockdiag W2: A covers output rows 0:50 (h=0), B rows 50:100 (h=1)
    wbdf = np.zeros((P1, 2 * P2), np.float32)
    for blk, h in ((0, 0), (1, 1)):
        off = blk * P2 + h * G * H2
        for g in range(G):
            wbdf[g * H1:(g + 1) * H1, off + g * H2:off + (g + 1) * H2] = W2.T
    A, Bm = wbdf[:, :P2], wbdf[:, P2:]
    Aa, Ab = _f16_split(A)
    Ba, Bb = _f16_split(Bm)
    ins["wbd"] = np.concatenate([Aa, Ab, Ba, Bb], axis=1)
    ins["negI"] = (-np.eye(P2)).astype(np.float16)

    # per-step sigma2 threshold columns: th2[:, t] = cb2*s_t - 2
    cb2 = (W2.astype(np.float64).sum(axis=1) + 2.0 * b2.astype(np.float64)
           - 1.0)
    cb2_row = np.tile(cb2, 2 * G)  # [100] matching rec rows
    s = np.cumsum(BETA ** np.arange(T))  # s_t for t=0..T-1
    th2 = (cb2_row[:, None] * s[None, :] - 2.0).astype(np.float32)
    ins["th2"] = th2
    return ins


def kernel(x, W1, b1, W2, b2, _variant="v4", _trace=False, _tmpdir=None):
    from concourse.bass_utils import run_bass_kernel_spmd

    x = np.asarray(x, np.float32)
    W1 = np.asarray(W1, np.float32)
    b1 = np.asarray(b1, np.float32)
    W2 = np.asarray(W2, np.float32)
    b2 = np.asarray(b2, np.float32)
    B = x.shape[0]
    assert B == B_FULL, f"kernel hardcoded for B={B_FULL}, got {B}"

    full = _host_inputs(x, W1, b1, W2, b2)
    shared = {k: v for k, v in full.items() if k != "xq"}
    in_maps = []
    for c in range(N_CORES):
        m = dict(shared)
        m["xq"] = np.ascontiguousarray(
            full["xq"][:, c].reshape(D, 2 * G * COLS))
        in_maps.append(m)

    nc = _get_nc(_variant)
    res = run_bass_kernel_spmd(
        nc,
        in_maps,
        core_ids=list(range(N_CORES)),
        trace=_trace,
        tmpdir=_tmpdir,
    )

    # host post: mem2 = (M'' + cb2*s_t)/2 ; spk2 = (M'' + th2[:,t]) > 0
    cb2 = (W2.astype(np.float64).sum(axis=1) + 2.0 * b2.astype(np.float64)
           - 1.0)
    cb2_row = np.tile(cb2, 2 * G).astype(np.float32)  # [100]
    s = np.cumsum(np.float64(BETA) ** np.arange(T))
    th2 = (cb2_row[:, None].astype(np.float64) * s[None, :]
           - 2.0).astype(np.float32)  # [100, T]

    spk = np.empty((T, N_CORES * BC, H2), np.float32)
    mem = np.empty((T, N_CORES * BC, H2), np.float32)
    for c in range(N_CORES):
        r = res.results[c]["mrec"]  # [T, 100, 410] fp32 M''
        # replicate the device Sign in fp32, then add back the bias traj
        z = r + th2.T[:, :, None]  # fp32 [T, 100, 410]
        spk_c = (z > 0.0).astype(np.float32)
        mem_c = (r.astype(np.float64)
                 + (cb2_row[:, None].astype(np.float64) * s[None, :]).T[
                     :, :, None]) * 0.5
        for q, dst in ((spk_c, spk), (mem_c.astype(np.float32), mem)):
            qq = q.reshape(T, 2, G, H2, HALF)     # [t, h, g, j, n]
            qq = qq.transpose(0, 2, 1, 4, 3)      # [t, g, h, n, j]
            dst[:, c * BC:(c + 1) * BC, :] = qq.reshape(T, BC, H2)
    spk = spk[:, :B_FULL, :]
    mem = mem[:, :B_FULL, :]
    kernel._last_results = res
    return spk, mem
